# revision 1
# baseline (speedup 1.0000x reference)
"""Causal self-attention (B=2, N=2048, D=1024, H=16, hd=64) on 8 trn2 NeuronCores.

Sharding: core c handles batch b = c//4 and 4 heads hs = [4*(c%4) .. 4*(c%4)+3]
(tensor-parallel over heads x data-parallel over batch). Each core computes its
heads' attention and a row-parallel partial of the output projection
(partial[n, :] = sum_{local heads} sa_h[n, :] @ Wo[h*64:(h+1)*64, :]); the host
sums the 4 partials per batch and adds the output bias.

Device algorithm per core (all matmuls fp32r = full-rate fp32):
  - qkT[col, n] = (x @ Wqk).T computed column-major so kT/qT land with head_dim
    on partitions; head pairs are stacked on partitions 0:64 / 64:128 so the two
    K=64 score matmuls run concurrently in the PE array (row tiling).
  - v65[n, 65] per head: v columns plus a ones column, so the attn@v matmul's
    row 64 accumulates the softmax denominator for free.
  - scoresT[nk, nq] tiles -> Exp on ScalarE (scale=1/8 folded in) -> causal via
    partial-width tiles + triangular mask on the diagonal 128x128 block ->
    attn@v accumulation in PSUM -> normalize by broadcasted reciprocal row sum.
"""

import numpy as np
from contextlib import ExitStack

import concourse.bass as bass
import concourse.tile as tile
from concourse import bacc, mybir
from concourse import bass_utils

F32 = mybir.dt.float32
F32R = mybir.dt.float32r
BF16 = mybir.dt.bfloat16
EXP = mybir.ActivationFunctionType.Exp

B, N, D, H, HD = 2, 2048, 1024, 16, 64
N_CORES = 8
LH = 4            # local heads per core
KT = D // 128     # 8 contraction k-tiles
NT = N // 128     # 16 n-tiles
NB = N // 512     # 4 n-blocks / q-blocks
QB = 512

_CACHE: dict = {}

# tuning knobs (A/B experiments)
CFG = {
    "acc_split": True,    # per-head 1-bank accumulators instead of paired 2-bank
    "at_bufs": 4,
    "oe_bufs": 4,
    "sc_bufs": 2,
    "late_outproj": True,
    "bf16_attn": False,
    "skip_attention": False,
    "skip_outproj": False,
    "xt_dma_split": True,
    "weave_outproj": False,
    "act_primer": True,
    "weave_last": False,
    "early_attn": True,
}


def _emit(nc, tc, ctx, io, repeat=1, dbg=None):
    xT, wqk, wv, bqk, bv, wo, tri, out = io

    persist = ctx.enter_context(tc.tile_pool(name="persist", bufs=1))
    sbp = ctx.enter_context(tc.tile_pool(name="work", bufs=1))
    psum = ctx.enter_context(tc.tile_pool(name="psum", bufs=1, space="PSUM"))

    # ---- persistent SBUF tensors ----
    xT_sb = persist.tile([128, KT, N], F32R)
    wqk_sb = persist.tile([128, KT, 512], F32R)
    wv_sb = persist.tile([128, KT, 256], F32R)
    wo_sb = persist.tile([128, 2, 1024], F32R)
    bqk_sb = persist.tile([128, 4], F32)
    bv_sb = persist.tile([1, 256], F32R)
    ones_sb = persist.tile([1, 128], F32R)
    tri_sb = persist.tile([128, 128], F32R)
    # (tri stays f32r; DVE converts on read)
    qkT_sb = persist.tile([128, 4, N], F32R)
    at_dt = BF16 if CFG["bf16_attn"] else F32R
    v65_sb = persist.tile([128, NT, LH * 65], at_dt)
    saT_sb = persist.tile([128, 2, N], F32R)

    # ---- input DMAs ----
    for kt in range(KT):
        nc.sync.dma_start(wqk_sb[:, kt, :], wqk[kt * 128:(kt + 1) * 128, :])
        nc.sync.dma_start(wv_sb[:, kt, :], wv[kt * 128:(kt + 1) * 128, :])
    nc.sync.dma_start(bqk_sb[:], bqk.rearrange("t p -> p t"))
    nc.sync.dma_start(bv_sb[:], bv[:])
    nc.sync.dma_start(tri_sb[:], tri[:])
    if CFG["xt_dma_split"]:
        for nb in range(NB):
            for kt in range(KT):
                nc.sync.dma_start(xT_sb[:, kt, nb * QB:(nb + 1) * QB],
                                  xT[kt * 128:(kt + 1) * 128, nb * QB:(nb + 1) * QB])
    else:
        for kt in range(KT):
            nc.sync.dma_start(xT_sb[:, kt, :], xT[kt * 128:(kt + 1) * 128, :])
    for kt2 in range(2):
        nc.sync.dma_start(wo_sb[:, kt2, :], wo[kt2 * 128:(kt2 + 1) * 128, :])
    nc.vector.memset(ones_sb[:].bitcast(F32), 1.0)
    if CFG.get("act_primer", True):
        # load the exp table set before the first real activation needs it
        primer = sbp.tile([1, 1], F32, name="t_primer", tag="primer", bufs=1)
        nc.scalar.activation(primer[:], ones_sb[0:1, 0:1].bitcast(F32), EXP)
    if CFG["bf16_attn"]:
        nc.vector.memset(v65_sb[:], 1.0)
    else:
        nc.vector.memset(v65_sb[:].bitcast(F32), 1.0)

    # ---- phase helpers ----
    def v_proj(nt):
        ps = psum.tile([128, 512], F32, name="ps_pj", tag="op", bufs=2)[:, 0:256]
        for kt in range(KT):
            nc.tensor.matmul(
                ps[:], xT_sb[:, kt, nt * 128:(nt + 1) * 128], wv_sb[:, kt, :],
                start=(kt == 0), stop=False,
            )
        nc.tensor.matmul(ps[:], ones_sb[:], bv_sb[:], start=False, stop=True)
        nc.vector.tensor_copy(
            v65_sb[:, nt, :].rearrange("p (h c) -> p h c", c=65)[:, :, 0:64],
            ps[:, :].rearrange("p (h c) -> p h c", c=64),
        )

    def qk_proj(nb):
        for ct in range(4):
            ps = psum.tile([128, 512], F32, name="ps_pj", tag="op", bufs=2)
            for kt in range(KT):
                nc.tensor.matmul(
                    ps[:], wqk_sb[:, kt, ct * 128:(ct + 1) * 128],
                    xT_sb[:, kt, nb * QB:(nb + 1) * QB],
                    start=(kt == 0), stop=(kt == KT - 1),
                )
            nc.vector.tensor_scalar_add(
                qkT_sb[:, ct, nb * QB:(nb + 1) * QB], ps[:], bqk_sb[:, ct:ct + 1]
            )

    def attention(J, weave=None):
        if CFG["skip_attention"]:
            return
        for p in range(2):            # head pairs (2p, 2p+1)
            if weave is not None and p == 1:
                weave()
            # one 2-bank accumulator per pair: head s in columns s*512:(s+1)*512
            if CFG["acc_split"]:
                acc0 = psum.tile([128, 512], F32, name="ps_acc", tag="acc", bufs=2)
                acc1 = psum.tile([128, 512], F32, name="ps_acc", tag="acc", bufs=2)
                accv = lambda s: (acc0, acc1)[s]
                accslice = lambda s, a, b: accv(s)[a, b]
            else:
                acc = psum.tile([128, 1024], F32, name="ps_acc", tag="acc", bufs=1)
            n_t = 4 * J + 4
            for t in range(n_t):
                d = t - 4 * J
                c0 = max(d, 0) * 128
                # both heads of the pair share one 2-bank PSUM tile; the two
                # K=64 matmuls use partition halves -> run concurrently in PE
                sc = psum.tile([128, 1024], F32, name="ps_sc", tag="sc", bufs=CFG["sc_bufs"])
                for s in range(2):
                    nc.tensor.matmul(
                        sc[:, s * 512 + c0:(s + 1) * 512],
                        qkT_sb[s * 64:(s + 1) * 64, 2 * p, t * 128:(t + 1) * 128],
                        qkT_sb[s * 64:(s + 1) * 64, 2 * p + 1, J * QB + c0:(J + 1) * QB],
                        start=True, stop=True,
                    )
                # one fused Exp for both heads ([512:512+c0] is unread garbage)
                at = sbp.tile([128, 1024], at_dt, name="t_at", tag="at", bufs=CFG["at_bufs"])
                nc.scalar.activation(at[:, c0:1024], sc[:, c0:1024], EXP, scale=0.125)
                if d >= 0:
                    # fused causal tri-mask for both heads (stride-512 view)
                    atv = at[:, c0:c0 + 640].rearrange(
                        "p (s c) -> p s c", c=128)[:, ::4, :]
                    nc.vector.tensor_mul(
                        atv, atv, tri_sb[:, None, :].broadcast_to([128, 2, 128])
                    )
                for s in range(2):
                    h = 2 * p + s
                    dst = (accv(s)[0:65, c0:512] if CFG["acc_split"]
                           else acc[0:65, s * 512 + c0:(s + 1) * 512])
                    nc.tensor.matmul(
                        dst,
                        v65_sb[:, t, h * 65:h * 65 + 65],
                        at[:, s * 512 + c0:(s + 1) * 512],
                        start=(t == 0), stop=(t == n_t - 1),
                        skip_group_check=True,
                    )
            if CFG["acc_split"]:
                for s in range(2):
                    h = 2 * p + s
                    rc = sbp.tile([1, 512], F32, name="t_rc", tag="rc", bufs=CFG.get("rc_bufs", 2))
                    nc.vector.reciprocal(rc[:], accv(s)[64:65, :])
                    bc = sbp.tile([64, 512], F32, name="t_bc", tag="bc", bufs=CFG.get("rc_bufs", 2))
                    nc.gpsimd.partition_broadcast(bc[:], rc[:])
                    po = (h % 2) * 64
                    nc.vector.tensor_mul(
                        saT_sb[po:po + 64, h // 2, J * QB:(J + 1) * QB],
                        accv(s)[0:64, :], bc[:],
                    )
            else:
                rc = sbp.tile([1, 1024], F32, name="t_rc", tag="rc", bufs=2)
                nc.vector.reciprocal(rc[:], acc[64:65, :])
                bc = sbp.tile([64, 1024], F32, name="t_bc", tag="bc", bufs=2)
                nc.gpsimd.partition_broadcast(bc[:], rc[:])
                if dbg is not None and J == 0:
                    nc.sync.dma_start(dbg["acc"][p * 128:p * 128 + 128, :],
                                      _dbg_copy(acc[:, :]))
                    nc.sync.dma_start(dbg["rc"][p:p + 1, :], rc[:])
                    nc.sync.dma_start(dbg["bc"][p * 64:(p + 1) * 64, :], bc[:])
                for s in range(2):
                    h = 2 * p + s
                    po = (h % 2) * 64
                    nc.vector.tensor_mul(
                        saT_sb[po:po + 64, h // 2, J * QB:(J + 1) * QB],
                        acc[0:64, s * 512:(s + 1) * 512], bc[:, s * 512:(s + 1) * 512],
                    )

    def out_proj(J):
        if CFG["skip_outproj"] or CFG["skip_attention"]:
            return
        for nqs in range(4):
            r0 = J * QB + nqs * 128
            for dh in range(2):
                op = psum.tile([128, 512], F32, name="ps_op", tag="op", bufs=2)
                for kt2 in range(2):
                    nc.tensor.matmul(
                        op[:],
                        saT_sb[:, kt2, r0:r0 + 128],
                        wo_sb[:, kt2, dh * 512:(dh + 1) * 512],
                        start=(kt2 == 0), stop=(kt2 == 1),
                    )
                oe = sbp.tile([128, 512], F32, name="t_oe", tag="oe", bufs=CFG["oe_bufs"])
                if CFG.get("oe_alt_engine", False) and (nqs + dh) % 2 == 1:
                    nc.scalar.copy(oe[:], op[:])
                else:
                    nc.vector.tensor_copy(oe[:], op[:])
                nc.sync.dma_start(out[r0:r0 + 128, dh * 512:(dh + 1) * 512], oe[:])

    def _dbg_copy(ps_ap):
        t = sbp.tile([128, 1024], F32, name="t_dbgc", tag="dbgc", bufs=2)
        nc.vector.tensor_copy(t[0:ps_ap.shape[0], :], ps_ap)
        return t[0:ps_ap.shape[0], :]

    # ---- optional stage dumps for debugging ----
    def dump_stages():
        if dbg is None:
            return
        for kt2 in range(2):
            nc.sync.dma_start(dbg["saT"][kt2 * 128:(kt2 + 1) * 128, :],
                              saT_sb[:, kt2, :].bitcast(F32))
        for ct in range(4):
            nc.sync.dma_start(dbg["qkT"][ct * 128:(ct + 1) * 128, :],
                              qkT_sb[:, ct, :].bitcast(F32))
        nc.sync.dma_start(dbg["v65"][:, :],
                          v65_sb[:, :, :].rearrange("p a b -> p (a b)").bitcast(F32))

    # ---- emission order: overlap projections with attention ----
    for _rep in range(repeat):
        if CFG["weave_outproj"]:
            for nt in range(4):
                v_proj(nt)
            qk_proj(0)
            for nt in range(4, 8):
                v_proj(nt)
            qk_proj(1)
            attention(0)
            for nt in range(8, 12):
                v_proj(nt)
            qk_proj(2)
            attention(1, weave=lambda: out_proj(0))
            for nt in range(12, 16):
                v_proj(nt)
            qk_proj(3)
            attention(2, weave=lambda: out_proj(1))
            attention(3, weave=lambda: out_proj(2))
            out_proj(3)
        elif CFG.get("early_attn", False):
            for nt in range(4):
                v_proj(nt)
            qk_proj(0)
            attention(0)
            for nt in range(4, 8):
                v_proj(nt)
            qk_proj(1)
            attention(1)
            for nt in range(8, 12):
                v_proj(nt)
            qk_proj(2)
            if CFG.get("outp_pre_attn", False):
                out_proj(0)
                attention(2)
                for nt in range(12, 16):
                    v_proj(nt)
                qk_proj(3)
                out_proj(1)
                attention(3)
                out_proj(2)
                out_proj(3)
            else:
                attention(2)
                out_proj(0)
                for nt in range(12, 16):
                    v_proj(nt)
                qk_proj(3)
                out_proj(1)
                attention(3)
                out_proj(2)
                out_proj(3)
        elif CFG["late_outproj"]:
            for nt in range(4):
                v_proj(nt)
            qk_proj(0)
            for nt in range(4, 8):
                v_proj(nt)
            qk_proj(1)
            attention(0)
            for nt in range(8, 12):
                v_proj(nt)
            qk_proj(2)
            attention(1)
            out_proj(0)
            for nt in range(12, 16):
                v_proj(nt)
            qk_proj(3)
            attention(2)
            out_proj(1)
            attention(3, weave=(lambda: out_proj(2)) if CFG.get("weave_last", True) else None)
            if not CFG.get("weave_last", True):
                out_proj(2)
            out_proj(3)
        else:
            for nt in range(4):
                v_proj(nt)
            qk_proj(0)
            for nt in range(4, 8):
                v_proj(nt)
            qk_proj(1)
            attention(0)
            out_proj(0)
            for nt in range(8, 12):
                v_proj(nt)
            qk_proj(2)
            attention(1)
            out_proj(1)
            for nt in range(12, 16):
                v_proj(nt)
            qk_proj(3)
            attention(2)
            out_proj(2)
            attention(3)
            out_proj(3)
    dump_stages()


def build(repeat=1, debug=False):
    nc = bacc.Bacc("TRN2", target_bir_lowering=False, debug=False,
                   num_devices=N_CORES)
    xT = nc.dram_tensor("xT", [D, N], F32R, kind="ExternalInput").ap()
    wqk = nc.dram_tensor("wqk", [D, 512], F32R, kind="ExternalInput").ap()
    wv = nc.dram_tensor("wv", [D, 256], F32R, kind="ExternalInput").ap()
    bqk = nc.dram_tensor("bqk", [4, 128], F32, kind="ExternalInput").ap()
    bv = nc.dram_tensor("bv", [1, 256], F32R, kind="ExternalInput").ap()
    wo = nc.dram_tensor("wo", [256, 1024], F32R, kind="ExternalInput").ap()
    tri = nc.dram_tensor("tri", [128, 128], F32R, kind="ExternalInput").ap()
    out = nc.dram_tensor("out", [N, D], F32, kind="ExternalOutput").ap()
    dbg = None
    if debug:
        dbg = {
            "saT": nc.dram_tensor("dbg_saT", [256, N], F32, kind="ExternalOutput").ap(),
            "qkT": nc.dram_tensor("dbg_qkT", [512, N], F32, kind="ExternalOutput").ap(),
            "v65": nc.dram_tensor("dbg_v65", [128, NT * LH * 65], F32, kind="ExternalOutput").ap(),
            "acc": nc.dram_tensor("dbg_acc", [256, 1024], F32, kind="ExternalOutput").ap(),
            "rc": nc.dram_tensor("dbg_rc", [2, 1024], F32, kind="ExternalOutput").ap(),
            "bc": nc.dram_tensor("dbg_bc", [128, 1024], F32, kind="ExternalOutput").ap(),
        }

    with tile.TileContext(nc) as tc:
        with ExitStack() as ctx:
            _emit(nc, tc, ctx, (xT, wqk, wv, bqk, bv, wo, tri, out), repeat=repeat, dbg=dbg)
    nc.compile()
    return nc


def make_in_maps(x, Wqkv, bqkv, Wo):
    """Host-side sharding: per-core input dicts."""
    x = np.asarray(x, dtype=np.float32)
    Wqkv = np.asarray(Wqkv, dtype=np.float32)
    bqkv = np.asarray(bqkv, dtype=np.float32)
    Wo = np.asarray(Wo, dtype=np.float32)
    tri = np.triu(np.ones((128, 128), dtype=np.float32))
    in_maps = []
    for c in range(N_CORES):
        b, g = divmod(c, 4)
        hs = [4 * g + i for i in range(LH)]
        # source chunk order in Wqkv[h] columns: k (0:64), q (64:128), v (128:192)
        wqk_cols = []
        bqk_rows = []
        for p in range(2):
            hA, hB = hs[2 * p], hs[2 * p + 1]
            wqk_cols += [Wqkv[hA][:, 0:64], Wqkv[hB][:, 0:64]]    # k pair tile
            bqk_rows.append(np.concatenate([bqkv[hA][0:64], bqkv[hB][0:64]]))
            wqk_cols += [Wqkv[hA][:, 64:128], Wqkv[hB][:, 64:128]]  # q pair tile
            bqk_rows.append(np.concatenate([bqkv[hA][64:128], bqkv[hB][64:128]]))
        in_maps.append({
            "xT": np.ascontiguousarray(x[b].T),
            "wqk": np.ascontiguousarray(np.concatenate(wqk_cols, axis=1)),
            "wv": np.ascontiguousarray(
                np.concatenate([Wqkv[h][:, 128:192] for h in hs], axis=1)),
            "bqk": np.ascontiguousarray(np.stack(bqk_rows)),
            "bv": np.ascontiguousarray(
                np.concatenate([bqkv[h][128:192] for h in hs])[None, :]),
            "wo": np.ascontiguousarray(
                np.concatenate([Wo[h * HD:(h + 1) * HD, :] for h in hs], axis=0)),
            "tri": tri,
        })
    return in_maps


def kernel(x, Wqkv, bqkv, Wo, bo):
    if "nc" not in _CACHE:
        _CACHE["nc"] = build()
    nc = _CACHE["nc"]
    in_maps = make_in_maps(x, Wqkv, bqkv, Wo)
    res = bass_utils.run_bass_kernel_spmd(
        nc, in_maps, core_ids=list(range(N_CORES)))
    bo = np.asarray(bo, dtype=np.float32)
    full = np.empty((B, N, D), dtype=np.float32)
    for b in range(B):
        acc = res.results[4 * b]["out"].astype(np.float32).copy()
        for g in range(1, 4):
            acc += res.results[4 * b + g]["out"]
        full[b] = acc + bo[None, :]
    return full

